# revision 16
# baseline (speedup 1.0000x reference)
"""Trainium2 Bass kernel for nn_GCDDLayer (Gaussian-curvature diffusion layer).

Math (per 512x512 image, zero-padded 3x3 convs):
    ux  = conv(u, SOBEL_X);  uy  = conv(u, SOBEL_Y)
    uxx = conv(ux, SOBEL_X); uxy = conv(ux, SOBEL_Y); uyy = conv(uy, SOBEL_Y)
    G   = (uxx*uyy - uxy^2) / ((1 + ux^2 + uy^2)^2 + 1e-6)
    phi = exp(-|G|); P = phi*ux; Q = phi*uy
    out = u + conv(P, SOBEL_X) + conv(Q, SOBEL_Y)

Strategy: pure data parallel over batch (16 samples -> 8 cores x 2 samples),
each core processes 6 independent 512x512 images (2 samples x 3 channels),
each cut into 5 overlapping 128-row tiles (stride 122; 3-row halo absorbs the
3-deep conv chain). Convs run on the TensorEngine as banded-matrix matmuls
(y-direction via the band, x-direction via shifted column reads of zero-padded
SBUF tiles, accumulated in PSUM).

v2 changes vs v1 (226us):
- bf16 for u and all SBUF intermediates: 2x DVE tensor_tensor, halved DMA-in.
  Verified vs the fp32 jax reference: rel err ~1e-2 < 2e-2 budget.
- residual `u +` folded into the stage-C PSUM accumulation via an identity
  band matmul; output DMA'd straight from PSUM (saves a [128,512] DVE add and
  an SBUF tile per tile).
- x-derivative prefixes d=Dx(u), dP=Dx(P) computed on the idle GPSIMD/Pool
  engine, turning conv(.,SOBEL_X) from 2 matmuls into 1 (17 MMs/tile incl.
  identity vs 18 in v1).
- |num| via DVE tensor_scalar(abs_max, 0) instead of ACT Abs; ACT keeps only
  Ln/Exp/Exp (one table set) + evacs.
"""

import os

import numpy as np

B, C, H, W = 16, 3, 512, 512
N_CORES = 8
IMGS = (B // N_CORES) * C  # 6 images per core
PAD = 3
BLK = W + 2 * PAD  # 518
NT = 5  # row tiles per image
TILE_STARTS = [0, 122, 244, 366, 384]
OUT_ROWS = [(0, 125), (125, 247), (247, 369), (369, 491), (491, 512)]
WIDTH = NT * BLK  # 2590
PWIDTH = NT * W  # 2560

_CACHE = {}


def _split_multiwaits(nc):
    """Walrus in this container accepts only one sync-wait per instruction;
    Tile emits multi-wait instructions. Split: for an instruction with k>1
    waits, insert k-1 single-wait NoOps before it on the same engine (engine
    queues are strict FIFO, so sequential waiting is equivalent)."""
    import concourse.mybir as mybir

    ctr = [0]

    def fresh(base):
        ctr[0] += 1
        return f"{base}-wsplit{ctr[0]}"

    for f in nc.m.functions:
        for b in f.blocks:
            changed = False
            newlist = []
            for ins in b.instructions:
                si = ins.sync_info
                if si is not None and len(si.on_wait) > 1:
                    waits = list(si.on_wait)
                    for w in waits[:-1]:
                        newlist.append(
                            mybir.InstNoOp(
                                name=fresh(ins.name),
                                engine=ins.engine,
                                debug=ins.debug,
                                ins=[],
                                outs=[],
                                sync_info=mybir.SyncInfo(on_wait=[w], on_update=[]),
                            )
                        )
                    ins.sync_info = mybir.SyncInfo(
                        on_wait=[waits[-1]], on_update=list(si.on_update)
                    )
                    changed = True
                newlist.append(ins)
            if changed:
                b.instructions = newlist


def _band(c0, c1, c2, n=128):
    # lhsT[k, m] = col[k - m + 1] (k: input row partition, m: output row)
    return (
        np.diag(np.full(n, c1))
        + np.diag(np.full(n - 1, c0), 1)
        + np.diag(np.full(n - 1, c2), -1)
    ).astype(np.float32)


def _bands_np():
    a = _band(1, 2, 1)
    return np.stack(
        [
            a,  # BSp: SOBEL_X col dx=+1 (also the A smoothing band)
            _band(-1, -2, -1),  # BSm: SOBEL_X col dx=-1
            _band(-1, 0, 1),  # BD : SOBEL_Y col dx=+-1
            _band(-2, 0, 2),  # BD2: SOBEL_Y col dx=0
            np.eye(128, dtype=np.float32),  # IDT: residual u
            (a @ a).astype(np.float32),  # A2: y-smooth twice (pentadiagonal)
        ]
    )


def _build():
    import concourse.bass as bass
    import concourse.mybir as mybir
    import concourse.tile as tile

    f32 = mybir.dt.float32
    bf16 = mybir.dt.bfloat16
    AF = mybir.ActivationFunctionType
    ALU = mybir.AluOpType

    # knobs
    pool_mode = int(os.environ.get("GCDD_POOL", "2"))  # 0 none, 1 dP, 2 d+dP
    uxx_act_tiles = int(os.environ.get("GCDD_UXX_ACT", "2"))  # uxx evac on ACT for t < this
    out_act_tiles = int(os.environ.get("GCDD_OUT_ACT", "2"))  # out evac via idt-MM+ACT for t < this
    psa_bufs = int(os.environ.get("GCDD_PSA_BUFS", "2"))
    psdiv_bufs = int(os.environ.get("GCDD_PSDIV_BUFS", "1"))
    sub_pool = os.environ.get("GCDD_SUB_POOL", "1") == "1"  # num sub on Pool
    ag_pool = os.environ.get("GCDD_AG_POOL", "1") == "1"  # aG mul on Pool
    q_pool = os.environ.get("GCDD_Q_POOL", "0") == "1"  # q stt on Pool

    nc = bass.Bass()
    u_dram = nc.dram_tensor("u", [IMGS, H, W], bf16, kind="ExternalInput")
    bands_dram = nc.dram_tensor("bands", [6, 128, 128], bf16, kind="ExternalInput")
    out_dram = nc.dram_tensor("out", [IMGS, H, W], bf16, kind="ExternalOutput")

    with tile.TileContext(nc) as tc:
        with (
            tc.tile_pool(name="const", bufs=1) as cpool,
            tc.tile_pool(name="pad", bufs=1) as ppool,
            tc.tile_pool(name="pad2", bufs=2) as ppool2,
            tc.tile_pool(name="upad", bufs=2) as upool,
            tc.tile_pool(name="dx", bufs=2) as dpool,
            tc.tile_pool(name="plain2", bufs=2) as spool2,
            tc.tile_pool(name="psum_a", bufs=psa_bufs, space="PSUM") as qpool_a,
            tc.tile_pool(name="psum_b", bufs=1, space="PSUM") as qpool_b,
            tc.tile_pool(name="psum_d", bufs=psdiv_bufs, space="PSUM") as qpool_d,
        ):
            bsp = cpool.tile([128, 128], bf16, tag="bsp")
            bsm = cpool.tile([128, 128], bf16, tag="bsm")
            bd = cpool.tile([128, 128], bf16, tag="bd")
            bd2 = cpool.tile([128, 128], bf16, tag="bd2")
            idt = cpool.tile([128, 128], bf16, tag="idt")
            a2 = cpool.tile([128, 128], bf16, tag="a2")
            for j, b_ in enumerate((bsp, bsm, bd, bd2, idt, a2)):
                nc.sync.dma_start(out=b_[:], in_=bands_dram[j])

            def mm_sx(ps, src, t, start=True, stop=True):
                # conv columns of SOBEL_X: dx=-1 -> BSm, dx=+1 -> BSp
                base = BLK * t + PAD
                for j, (b_, dx) in enumerate(((bsm, -1), (bsp, +1))):
                    nc.tensor.matmul(
                        ps[:],
                        b_[:],
                        src[:, base + dx : base + dx + W],
                        start=(j == 0) and start,
                        stop=(j == 1) and stop,
                    )

            def mm_sy(ps, src, t, start=True, stop=True):
                # conv columns of SOBEL_Y: dx=-1 -> BD, 0 -> BD2, +1 -> BD
                base = BLK * t + PAD
                for j, (b_, dx) in enumerate(((bd, -1), (bd2, 0), (bd, +1))):
                    nc.tensor.matmul(
                        ps[:],
                        b_[:],
                        src[:, base + dx : base + dx + W],
                        start=(j == 0) and start,
                        stop=(j == 2) and stop,
                    )

            import contextlib
            reps = int(os.environ.get("GCDD_REPS", "0"))
            loop_cm = tc.For_i(0, reps) if reps > 1 else contextlib.nullcontext()
            with loop_cm:
              for i in range(IMGS):
                u_pad = upool.tile([128, WIDTH], bf16, tag="u")
                uxuy_pad = ppool2.tile([128, 2 * WIDTH], bf16, tag="uxuy")
                pq_pad = ppool.tile([128, 2 * WIDTH], bf16, tag="pq")
                uxxs = spool2.tile([128, PWIDTH], bf16, tag="uxxs")
                sqxy = spool2.tile([128, PWIDTH], bf16, tag="sqxy")
                ta = spool2.tile([128, PWIDTH], bf16, tag="ta")
                tb = spool2.tile([128, PWIDTH], bf16, tag="tb")
                tnum = spool2.tile([128, PWIDTH], bf16, tag="tnum")
                outs = spool2.tile([128, PWIDTH], bf16, tag="outs")
                if pool_mode >= 3:
                    dxu = dpool.tile([128, WIDTH], bf16, tag="dxu")
                    dxd2 = dpool.tile([128, PWIDTH], bf16, tag="dxd2")
                elif pool_mode >= 2:
                    dxu = dpool.tile([128, PWIDTH], bf16, tag="dxu")
                if pool_mode >= 1:
                    dxp = dpool.tile([128, PWIDTH], bf16, tag="dxp")

                # zero the x-halo pad columns of every shifted-read tensor
                # (pads are never overwritten afterwards, so only fresh pool
                # slots need it: bufs=2 tags on images 0/1, bufs=1 on image 0)
                fresh2 = (u_pad, uxuy_pad) if i < 2 else ()
                fresh2 += (dxu,) if (pool_mode >= 3 and i < 2) else ()
                fresh1 = (pq_pad,) if i == 0 else ()
                for t_ in fresh2 + fresh1:
                    v = t_[:].rearrange("p (n b) -> p n b", b=BLK)
                    nc.vector.memset(v[:, :, 0:PAD], 0)
                    nc.vector.memset(v[:, :, PAD + W : BLK], 0)

                # 3D views
                u3 = u_pad[:].rearrange("p (n b) -> p n b", b=BLK)
                uc = u3[:, :, PAD : PAD + W]
                uxuyv = uxuy_pad[:].rearrange("p (m n b) -> p m n b", m=2, b=BLK)
                uxuyc = uxuyv[:, :, :, PAD : PAD + W]
                ux3 = uxuy_pad[:, :WIDTH].rearrange("p (n b) -> p n b", b=BLK)
                uxc = ux3[:, :, PAD : PAD + W]
                uy3 = uxuy_pad[:, WIDTH:].rearrange("p (n b) -> p n b", b=BLK)
                uyc = uy3[:, :, PAD : PAD + W]
                pq3 = pq_pad[:].rearrange("p (m n b) -> p m n b", m=2, b=BLK)
                pqc = pq3[:, :, :, PAD : PAD + W]
                p3 = pq_pad[:, :WIDTH].rearrange("p (n b) -> p n b", b=BLK)
                uxx3 = uxxs[:].rearrange("p (n b) -> p n b", b=W)
                sqxy3 = sqxy[:].rearrange("p (n b) -> p n b", b=W)
                ta3 = ta[:].rearrange("p (n b) -> p n b", b=W)
                tb3 = tb[:].rearrange("p (n b) -> p n b", b=W)
                tnum3 = tnum[:].rearrange("p (n b) -> p n b", b=W)
                outs3 = outs[:].rearrange("p (n b) -> p n b", b=W)
                if pool_mode >= 3:
                    dxu3 = dxu[:].rearrange("p (n b) -> p n b", b=BLK)[:, :, PAD : PAD + W]
                    dxd23 = dxd2[:].rearrange("p (n b) -> p n b", b=W)
                elif pool_mode >= 2:
                    dxu3 = dxu[:].rearrange("p (n b) -> p n b", b=W)
                if pool_mode >= 1:
                    dxp3 = dxp[:].rearrange("p (n b) -> p n b", b=W)

                # load u tiles
                for t in range(NT):
                    st = TILE_STARTS[t]
                    nc.sync.dma_start(
                        out=u_pad[:, BLK * t + PAD : BLK * t + PAD + W],
                        in_=u_dram[i, st : st + 128, :],
                    )

                # ---- stage A: first derivatives -------------------------
                for t in range(NT):
                    base = BLK * t + PAD
                    ps_a = qpool_a.tile([128, 2 * W], f32, tag="ps_a")
                    if pool_mode >= 2:
                        # d = Dx(u) on Pool; ux = A @ d (1 matmul)
                        nc.gpsimd.tensor_sub(
                            dxu3[:, t, :],
                            u_pad[:, base + 1 : base + 1 + W],
                            u_pad[:, base - 1 : base - 1 + W],
                        )
                        nc.tensor.matmul(
                            ps_a[:, :W], bsp[:], dxu3[:, t, :], start=True, stop=True
                        )
                    else:
                        mm_sx(ps_a[:, :W], u_pad, t)
                    mm_sy(ps_a[:, W:], u_pad, t)
                    nc.scalar.copy(
                        uxuyc[:, :, t, :],
                        ps_a[:].rearrange("p (m w) -> p m w", m=2),
                    )

                # ---- stage B: second derivatives ------------------------
                for t in range(NT):
                    base = BLK * t + PAD
                    ps_uxx = qpool_b.tile([128, W], f32, tag="ps_uxx")
                    if pool_mode >= 3:
                        # d2 = Dx(d) on Pool; uxx = A^2 @ d2 (1 matmul)
                        nc.gpsimd.tensor_sub(
                            dxd23[:, t, :],
                            dxu[:, base + 1 : base + 1 + W],
                            dxu[:, base - 1 : base - 1 + W],
                        )
                        nc.tensor.matmul(
                            ps_uxx[:], a2[:], dxd23[:, t, :], start=True, stop=True
                        )
                    else:
                        mm_sx(ps_uxx, uxuy_pad[:, :WIDTH], t)
                    if t < uxx_act_tiles:
                        nc.scalar.copy(uxx3[:, t, :], ps_uxx[:])
                    else:
                        nc.vector.tensor_copy(uxx3[:, t, :], ps_uxx[:])
                    ps_uxy = qpool_b.tile([128, W], f32, tag="ps_uxy")
                    mm_sy(ps_uxy, uxuy_pad[:, :WIDTH], t)
                    nc.scalar.square(sqxy3[:, t, :], ps_uxy[:])
                    ps_uyy = qpool_b.tile([128, W], f32, tag="ps_uyy")
                    mm_sy(ps_uyy, uxuy_pad[:, WIDTH:], t)
                    # nm = uxx * uyy (one PSUM operand max per DVE op)
                    nc.vector.tensor_mul(tnum3[:, t, :], ps_uyy[:], uxx3[:, t, :])

                # ---- pointwise chain (chunked so DVE/ACT pipeline) ------
                import json as _json
                _ck = _json.loads(os.environ.get("GCDD_CHUNKS", "[[0,1],[1,3],[3,5]]"))
                for lo, hi in _ck:
                    s = (slice(None), slice(lo, hi), slice(None))
                    nc.vector.tensor_mul(ta3[s], uxc[s], uxc[s])  # ux^2
                    nc.vector.tensor_mul(tb3[s], uyc[s], uyc[s])  # uy^2
                    (nc.gpsimd if q_pool else nc.vector).scalar_tensor_tensor(
                        ta3[s], ta3[s], 1.0, tb3[s], ALU.add, ALU.add
                    )  # q = (ux^2+1) + uy^2
                    # 1/q^2 = exp(-2 ln q); Ln/Exp share one ACT table set.
                    nc.scalar.activation(ta3[s], ta3[s], AF.Ln)
                    nc.scalar.activation(tb3[s], ta3[s], AF.Exp, scale=-2.0)
                    (nc.gpsimd if sub_pool else nc.vector).tensor_sub(
                        tnum3[s], tnum3[s], sqxy3[s]
                    )  # num = uxx*uyy - uxy^2
                    nc.vector.scalar_tensor_tensor(  # |num| = max(-num, num)
                        tnum3[s], tnum3[s], -1.0, tnum3[s], ALU.mult, ALU.max
                    )
                    (nc.gpsimd if ag_pool else nc.vector).tensor_mul(
                        tnum3[s], tnum3[s], tb3[s]
                    )  # aG
                    nc.scalar.activation(  # phi = exp(-aG)
                        tnum3[s], tnum3[s], AF.Exp, scale=-1.0
                    )
                    # P|Q = phi * (ux|uy) in one op (phi broadcast over m)
                    sm = (slice(None), slice(None), slice(lo, hi), slice(None))
                    nc.vector.tensor_mul(
                        pqc[sm],
                        tnum3[s].unsqueeze(1).broadcast_to((128, 2, hi - lo, W)),
                        uxuyc[sm],
                    )

                # ---- stage C: divergence + residual ---------------------
                for t in range(NT):
                    base = BLK * t + PAD
                    ps_div = qpool_d.tile([128, W], f32, tag="ps_div")
                    if pool_mode >= 1:
                        nc.gpsimd.tensor_sub(  # dP = Dx(P) on Pool
                            dxp3[:, t, :],
                            pq_pad[:, base + 1 : base + 1 + W],
                            pq_pad[:, base - 1 : base - 1 + W],
                        )
                        nc.tensor.matmul(  # div += A @ dP
                            ps_div[:], bsp[:], dxp3[:, t, :], start=True, stop=False
                        )
                    else:
                        mm_sx(ps_div, pq_pad[:, :WIDTH], t, start=True, stop=False)
                    st = TILE_STARTS[t]
                    lo, hi = OUT_ROWS[t]
                    if t < out_act_tiles:
                        # residual via identity band in PSUM, evac on ACT
                        mm_sy(ps_div, pq_pad[:, WIDTH:], t, start=False, stop=False)
                        nc.tensor.matmul(
                            ps_div[:],
                            idt[:],
                            u_pad[:, base : base + W],
                            start=False,
                            stop=True,
                        )
                        nc.scalar.copy(outs3[:, t, :], ps_div[:])
                    else:
                        mm_sy(ps_div, pq_pad[:, WIDTH:], t, start=False, stop=True)
                        nc.vector.tensor_add(outs3[:, t, :], ps_div[:], uc[:, t, :])
                    nc.sync.dma_start(
                        out=out_dram[i, lo:hi, :],
                        in_=outs[lo - st : hi - st, W * t : W * t + W],
                    )

    _split_multiwaits(nc)
    return nc


def _get_nc():
    if "nc" not in _CACHE:
        _CACHE["nc"] = _build()
    return _CACHE["nc"]


def make_in_maps(u):
    import ml_dtypes

    u = np.ascontiguousarray(u, dtype=np.float32).astype(ml_dtypes.bfloat16)
    bands = _bands_np().astype(ml_dtypes.bfloat16)
    per = B // N_CORES
    return [
        {
            "u": u[i * per : (i + 1) * per].reshape(IMGS, H, W),
            "bands": bands,
        }
        for i in range(N_CORES)
    ]


def kernel(u: np.ndarray, theta: np.ndarray = None) -> np.ndarray:
    from concourse.bass_utils import run_bass_kernel_spmd

    nc = _get_nc()
    in_maps = make_in_maps(u)
    res = run_bass_kernel_spmd(
        nc,
        in_maps,
        core_ids=list(range(N_CORES)),
        trace=os.environ.get("GCDD_TRACE", "0") == "1",
    )
    _CACHE["last_result"] = res
    per = B // N_CORES
    out = np.empty((B, C, H, W), np.float32)
    for i in range(N_CORES):
        out[i * per : (i + 1) * per] = (
            res.results[i]["out"].astype(np.float32).reshape(per, C, H, W)
        )
    return out
